# revision 56
# baseline (speedup 1.0000x reference)
"""GNN message-passing kernel for 8 trn2 NeuronCores (Bass/Tile).

Model (reference):
    msg  = relu(concat(x[src], x[dst], e_attr) @ W_msg + b_msg)   # [E, 30]
    x1   = segment_sum(msg, dst, N)                                # [N, 30]
    h    = relu(x1 @ W1 + b1)                                      # [N, 20]
    g    = segment_sum(h, batch, G)                                # [G, 20]
    out  = relu(g @ W2 + b2) @ W3 + b3                             # [G, 1]

Scheme (v2, "lane-aligned blocks"):
  Cores own whole graphs (G/8 graphs each -> contiguous node range, no
  cross-core node sharing, hence NO collectives).  Within a core, its
  nodes are sorted by in-degree and packed into blocks of 128 "lanes"
  in a "mountain" order (small blocks first and last, biggest in the
  middle, for a fast pipeline start and a short tail).  Block b gets
  T_b = max in-block degree tiles; tile t holds edge #t of every lane
  (zero-padded columns produce msg == 0 because the bias is folded
  into the dst-side node projection).  The per-edge matmul output
  partition therefore IS the destination node: no one-hot scatter
  matmuls, no gpsimd one-hot builds.  relu runs on the scalar (5/6)
  and vector (1/6) engines in 17-tile chunks; the per-node sum is a
  single DVE tensor_reduce per block.

  pass 1: P|Q' = [node_attr^T; 1]^T @ [W_src | W_dst; 0 | b]  (per-core
          node shard, streamed orientation: W stationary).
  host:   gathers P[src] + Q'[dst] (the "replicated node table" gather
          of the sharding strategy), sums them, and assembles the fp8
          per-edge stream sM [96, slots]: rows 0-29 P+Q', 30-93 e_attr.
  pass 2: per block: T_b msg matmuls (fp8 lhsT stream tile, rhs = Wc =
          [I30; W_e]) -> relu -> DVE reduce over tiles -> x1.  Per 4
          blocks: one PE transpose + one block-diagonal W1 matmul gives
          h for 512 nodes; per block one tiny one-hot (gpsimd
          local_scatter, 128 idxs) pools h into a per-core PSUM-resident
          gT [20, 128 graphs].  The graph head runs transposed (gT ->
          W2 -> W3) with no transposes and the core writes its own 125
          graphs; the host concatenates.
"""
import sys

if "/opt/trn_rl_repo" not in sys.path:
    sys.path.insert(0, "/opt/trn_rl_repo")

import numpy as np
import ml_dtypes

bf16 = ml_dtypes.bfloat16
f8 = ml_dtypes.float8_e4m3

NCORES = 8
DM = 30          # message dim
KS = 96          # stream rows (30 PQ + 64 e_attr + 2 pad)
CHT = 17         # msg tiles per relu chunk (17*30 f32 = 2040B <= 2KB bank)
GRP = 4          # blocks per W1 group


# ---------------------------------------------------------------- pass 1

def build_pq_program(npc_pad):
    import concourse.bacc as bacc
    import concourse.mybir as mybir
    import concourse.tile as tile
    from contextlib import ExitStack

    f32, bft = mybir.dt.float32, mybir.dt.bfloat16
    COPY = mybir.ActivationFunctionType.Copy
    NCH = npc_pad // 512

    nc = bacc.Bacc("TRN2", target_bir_lowering=False, debug=True)
    naT = nc.declare_dram_parameter("naT", [65, npc_pad], mybir.dt.float8e4, isOutput=False)
    Wpq = nc.declare_dram_parameter("Wpq", [65, 60], bft, isOutput=False)
    PQT = nc.declare_dram_parameter("PQT", [60, npc_pad], bft, isOutput=True)

    with tile.TileContext(nc) as tc, ExitStack() as xs:
        cp = xs.enter_context(tc.tile_pool(name="const", bufs=1))
        inp = xs.enter_context(tc.tile_pool(name="inp", bufs=16))
        outp = xs.enter_context(tc.tile_pool(name="outp", bufs=3))
        ps = xs.enter_context(tc.tile_pool(name="ps", bufs=3, space="PSUM"))

        Wpq_t = cp.tile([65, 60], bft)
        nc.scalar.dma_start(out=Wpq_t[:], in_=Wpq[:])
        # input chunks: small first chunk so the first matmul starts early
        bounds = [0, 512]
        while bounds[-1] < npc_pad:
            bounds.append(min(npc_pad, bounds[-1] + 1024))
        in_tiles = []
        dmae1 = [nc.sync, nc.gpsimd, nc.scalar]
        for i in range(len(bounds) - 1):
            lo, hi = bounds[i], bounds[i + 1]
            t = inp.tile([65, hi - lo], mybir.dt.float8e4, tag="nat")
            dmae1[i % 3].dma_start(out=t[:], in_=naT[:, lo:hi])
            in_tiles.append((t, lo, hi))

        def find_tile(lo):
            for t, tlo, thi in in_tiles:
                if tlo <= lo < thi:
                    return t, tlo, thi
            raise AssertionError(lo)

        pend = None
        for c in range(NCH):
            lo = c * 512
            ti, tlo, thi = find_tile(lo)
            pq_ps = ps.tile([60, 512], f32, tag="pq")
            nc.tensor.matmul(pq_ps[:], lhsT=Wpq_t[:],
                             rhs=ti[:, lo - tlo:lo - tlo + 512],
                             start=True, stop=True)
            if c % 2 == 0:
                ot = outp.tile([60, 1024], bft, tag="pqt")
                nc.scalar.activation(ot[:, 0:512], pq_ps[:], COPY)
                pend = (ot, lo)
            else:
                ot, olo = pend
                nc.vector.tensor_copy(out=ot[:, 512:1024], in_=pq_ps[:])
                eng = nc.sync if c % 4 == 1 else nc.gpsimd
                eng.dma_start(out=PQT[:, olo:olo + 1024], in_=ot[:])
                pend = None
        if pend is not None:
            ot, olo = pend
            nc.sync.dma_start(out=PQT[:, olo:olo + 512], in_=ot[:, 0:512])
    nc.finalize()
    return nc


# ---------------------------------------------------------------- pass 2

def build_main_program(nblk, tb, ngmax):
    """nblk: padded block count (multiple of GRP); tb: per-block tile
    counts (uniform across cores); ngmax: graphs per core (<=128)."""
    import concourse.bacc as bacc
    import concourse.mybir as mybir
    import concourse.tile as tile
    from contextlib import ExitStack

    f32, bft, fp8 = mybir.dt.float32, mybir.dt.bfloat16, mybir.dt.float8e4
    RELU = mybir.ActivationFunctionType.Relu
    COPY = mybir.ActivationFunctionType.Copy
    MAX = mybir.AluOpType.max
    ADD = mybir.AluOpType.add
    AXX = mybir.AxisListType.X

    nt = sum(tb)
    off = np.zeros(len(tb) + 1, np.int64)
    off[1:] = np.cumsum(tb)

    nc = bacc.Bacc("TRN2", target_bir_lowering=False, debug=True)
    sM = nc.declare_dram_parameter("sM", [KS, nt * 128], fp8, isOutput=False)
    Wc = nc.declare_dram_parameter("Wc", [KS, DM], fp8, isOutput=False)
    W1d = nc.declare_dram_parameter("W1d", [GRP * DM + 1, GRP * 20], bft,
                                    isOutput=False)
    W2a = nc.declare_dram_parameter("W2a", [21, 16], f32, isOutput=False)
    W3a = nc.declare_dram_parameter("W3a", [11, 16], f32, isOutput=False)
    ident = nc.declare_dram_parameter("ident", [128, 128], f32, isOutput=False)
    gidx = nc.declare_dram_parameter("gidx", [128, nblk * 2], mybir.dt.int16,
                                     isOutput=False)
    out = nc.declare_dram_parameter("out", [1, 128], f32, isOutput=True)

    with tile.TileContext(nc) as tc, ExitStack() as xs:
        cp = xs.enter_context(tc.tile_pool(name="const", bufs=1))
        sMp = xs.enter_context(tc.tile_pool(name="sMp", bufs=4))
        msgp = xs.enter_context(tc.tile_pool(name="msgp", bufs=3))
        x14p = xs.enter_context(tc.tile_pool(name="x14p", bufs=3))
        xTap = xs.enter_context(tc.tile_pool(name="xTap", bufs=2))
        h4p = xs.enter_context(tc.tile_pool(name="h4p", bufs=2))
        ohp = xs.enter_context(tc.tile_pool(name="ohp", bufs=2))
        ps_m = xs.enter_context(tc.tile_pool(name="ps_m", bufs=3, space="PSUM"))
        ps_t = xs.enter_context(tc.tile_pool(name="ps_t", bufs=2, space="PSUM"))
        ps_h = xs.enter_context(tc.tile_pool(name="ps_h", bufs=2, space="PSUM"))
        ps_g = xs.enter_context(tc.tile_pool(name="ps_g", bufs=1, space="PSUM"))

        # ---- constants
        Wc_t = cp.tile([KS, DM], fp8)
        nc.sync.dma_start(out=Wc_t[:], in_=Wc[:])
        W1d_t = cp.tile([GRP * DM + 1, GRP * 20], bft)
        nc.scalar.dma_start(out=W1d_t[:], in_=W1d[:])
        W2a_t = cp.tile([21, 16], f32)
        nc.scalar.dma_start(out=W2a_t[:], in_=W2a[:])
        W3a_t = cp.tile([11, 16], f32)
        nc.scalar.dma_start(out=W3a_t[:], in_=W3a[:])
        ident_t = cp.tile([128, 128], f32)
        nc.scalar.dma_start(out=ident_t[:], in_=ident[:])
        gidx_t = cp.tile([128, nblk * 2], mybir.dt.int16)
        nc.scalar.dma_start(out=gidx_t[:], in_=gidx[:])
        ones2 = cp.tile([128, 2], bft)
        nc.vector.memset(ones2[:], 1.0)

        gT_ps = ps_g.tile([20, 128], f32, tag="gT")

        dmae = [nc.sync, nc.gpsimd, nc.scalar]
        ngrp = nblk // GRP
        relu_i = 0
        dma_i = 0
        for g in range(ngrp):
            b0 = g * GRP
            gtiles = int(off[b0 + GRP] - off[b0])
            if gtiles > 0:
                smb = sMp.tile([KS, gtiles * 128], fp8, tag="sM")
                if g < 2:
                    # per-block DMAs so the first matmuls start early
                    for i in range(GRP):
                        c0 = int(off[b0 + i] - off[b0]) * 128
                        c1 = int(off[b0 + i + 1] - off[b0]) * 128
                        if c1 > c0:
                            dmae[dma_i % 2].dma_start(
                                out=smb[:, c0:c1],
                                in_=sM[:, int(off[b0]) * 128 + c0:
                                       int(off[b0]) * 128 + c1])
                            dma_i += 1
                else:
                    glo = int(off[b0]) * 128
                    ghi = int(off[b0 + GRP]) * 128
                    dmae[dma_i % 3].dma_start(out=smb[:], in_=sM[:, glo:ghi])
                    dma_i += 1
            x14 = x14p.tile([128, GRP * DM + 1], f32, tag="x14")
            nc.gpsimd.memset(x14[:, GRP * DM:GRP * DM + 1], 1.0)
            for i in range(GRP):
                b = b0 + i
                T = tb[b]
                if T == 0:
                    nc.gpsimd.memset(x14[:, i * DM:(i + 1) * DM], 0.0)
                    continue
                loc = int(off[b] - off[b0]) * 128
                x14s = x14[:, i * DM:(i + 1) * DM]
                msgblk = msgp.tile([128, T * DM], bft, tag="msg")
                for c0 in range(0, T, CHT):
                    cn = min(CHT, T - c0)
                    mps = ps_m.tile([128, cn * DM], f32, tag="mps")
                    for t in range(cn):
                        e0 = loc + (c0 + t) * 128
                        nc.tensor.matmul(
                            mps[:, t * DM:(t + 1) * DM],
                            lhsT=smb[:, e0:e0 + 128],
                            rhs=Wc_t[:], start=True, stop=True)
                    dst = msgblk[:, c0 * DM:(c0 + cn) * DM]
                    if relu_i % 8 < 7:
                        nc.scalar.activation(dst, mps[:], RELU)
                    else:
                        nc.vector.tensor_scalar(
                            out=dst, in0=mps[:], scalar1=0.0, scalar2=None,
                            op0=MAX)
                    relu_i += 1
                nc.vector.tensor_reduce(
                    out=x14s,
                    in_=msgblk[:].rearrange("p (t d) -> p d t", d=DM),
                    axis=AXX, op=ADD)
            # ---- W1 for the 4 blocks
            xT_ps = ps_t.tile([GRP * DM + 1, 128], f32, tag="xT")
            nc.tensor.transpose(out=xT_ps[:], in_=x14[:], identity=ident_t[:])
            xTa = xTap.tile([GRP * DM + 1, 128], bft, tag="xTa")
            nc.scalar.activation(xTa[:], xT_ps[:], COPY)
            h4_ps = ps_h.tile([128, GRP * 20], f32, tag="h4")
            nc.tensor.matmul(h4_ps[:], lhsT=xTa[:], rhs=W1d_t[:],
                             start=True, stop=True)
            h4 = h4p.tile([128, GRP * 20], bft, tag="h4s")
            nc.scalar.activation(h4[:], h4_ps[:], RELU)
            # ---- pool each block into gT
            for i in range(GRP):
                b = b0 + i
                oh = ohp.tile([128, 128], bft, tag="oh")
                nc.gpsimd.local_scatter(
                    out_ap=oh[:], data_ap=ones2[:],
                    idxs_ap=gidx_t[:, b * 2:b * 2 + 2],
                    channels=128, num_elems=128, num_idxs=2)
                nc.tensor.matmul(gT_ps[:], lhsT=h4[:, i * 20:(i + 1) * 20],
                                 rhs=oh[:], start=(b == 0), stop=(b == nblk - 1))

        # ---- graph head (per-core, transposed; no collectives)
        gTa = cp.tile([21, 128], f32)
        nc.vector.memset(gTa[:], 1.0)
        nc.vector.tensor_copy(out=gTa[:20, :], in_=gT_ps[:])
        r_ps = ps_m.tile([16, 128], f32, tag="mps")
        nc.tensor.matmul(r_ps[:], lhsT=W2a_t[:], rhs=gTa[:], start=True,
                         stop=True)
        rTa = cp.tile([11, 128], f32)
        nc.vector.memset(rTa[:], 1.0)
        nc.scalar.activation(rTa[:10, :], r_ps[:10, :], RELU)
        o_ps = ps_h.tile([16, 128], f32, tag="h4")
        nc.tensor.matmul(o_ps[:], lhsT=W3a_t[:], rhs=rTa[:], start=True,
                         stop=True)
        outsb = cp.tile([1, 128], f32)
        nc.vector.tensor_copy(out=outsb[:], in_=o_ps[:1, :])
        nc.sync.dma_start(out=out[:], in_=outsb[:])

    nc.finalize()
    return nc


# ---------------------------------------------------------------- host prep

def host_plan(edge_index, batch, n_nodes, n_graphs):
    """Graph-aligned core ownership + degree-sorted lane blocks."""
    src = np.asarray(edge_index[0]).astype(np.int64)
    dst = np.asarray(edge_index[1]).astype(np.int64)
    batch = np.asarray(batch).astype(np.int64)
    N, G = n_nodes, n_graphs

    gcnt = np.bincount(batch, minlength=G)
    gstart = np.zeros(G + 1, np.int64)
    gstart[1:] = np.cumsum(gcnt)
    glo = [c * G // NCORES for c in range(NCORES + 1)]
    nlo = [int(gstart[glo[c]]) for c in range(NCORES + 1)]

    deg = np.bincount(dst, minlength=N)

    order_e = np.argsort(dst, kind="stable")
    dst_s = dst[order_e]
    src_s = src[order_e]
    # per-edge rank within its dst run
    run_start = np.zeros(len(dst_s), np.int64)
    newrun = np.r_[True, dst_s[1:] != dst_s[:-1]]
    idx = np.arange(len(dst_s))
    run_start = np.maximum.accumulate(np.where(newrun, idx, 0))
    rank = idx - run_start

    # per-core sorted lanes
    cores = []
    nblk_max = 0
    for c in range(NCORES):
        lo, hi = nlo[c], nlo[c + 1]
        dg = deg[lo:hi]
        # "mountain" block order: small blocks first (fast pipeline
        # start) and last (short tail), biggest in the middle.  `oi` maps
        # lane position -> local node index (-1 = hole/pad lane).
        order_n = np.argsort(dg, kind="stable")
        nblk = (hi - lo + 127) // 128
        pad_n = nblk * 128 - (hi - lo)
        oi = np.concatenate([order_n, np.full(pad_n, -1, np.int64)])
        blocks = [oi[k * 128:(k + 1) * 128] for k in range(nblk)]
        oi = np.concatenate(blocks[0::2] + blocks[1::2][::-1])
        lane_of = np.full(hi - lo, -1, np.int64)
        pos = np.arange(nblk * 128)
        lane_of[oi[oi >= 0]] = pos[oi >= 0]
        nblk_max = max(nblk_max, nblk)
        dgp = np.where(oi >= 0, deg[lo + np.maximum(oi, 0)], 0)
        tbc = dgp.reshape(nblk, 128).max(axis=1)
        cores.append(dict(lo=lo, hi=hi, oi=oi, lane_of=lane_of,
                          tbc=tbc, glo=glo[c], ghi=glo[c + 1]))

    nblk = ((nblk_max + GRP - 1) // GRP) * GRP
    tb = np.zeros(nblk, np.int64)
    for cd in cores:
        tb[:len(cd["tbc"])] = np.maximum(tb[:len(cd["tbc"])], cd["tbc"])
    off = np.zeros(nblk + 1, np.int64)
    off[1:] = np.cumsum(tb)

    ngmax = max(cd["ghi"] - cd["glo"] for cd in cores)
    assert ngmax <= 128

    return dict(cores=cores, nblk=nblk, tb=tb, off=off, nt=int(tb.sum()),
                order_e=order_e, dst_s=dst_s, src_s=src_s, rank=rank,
                deg=deg, ngmax=ngmax, batch=batch)


def host_prep_pq(plan, node_attr, npc_pad):
    na = np.asarray(node_attr, np.float32)
    in_maps = []
    for cd in plan["cores"]:
        lo, hi = cd["lo"], cd["hi"]
        naT = np.zeros([65, npc_pad], f8)
        naT[0:64, :hi - lo] = na[lo:hi].T.astype(f8)
        naT[64, :] = f8(1.0)
        in_maps.append({"naT": naT})
    return in_maps


def make_weights(W_msg, b_msg, W1, b1, W2, b2, W3, b3):
    W_msg = np.asarray(W_msg, np.float32)
    Wpq = np.zeros([65, 60], np.float32)
    Wpq[0:64, 0:DM] = W_msg[0:64]
    Wpq[0:64, DM:2 * DM] = W_msg[64:128]
    Wpq[64, DM:2 * DM] = np.asarray(b_msg, np.float32)
    Wc = np.zeros([KS, DM], np.float32)
    Wc[0:DM] = np.eye(DM)
    Wc[DM:DM + 64] = W_msg[128:192]
    W1d = np.zeros([GRP * DM + 1, GRP * 20], np.float32)
    for i in range(GRP):
        W1d[i * DM:(i + 1) * DM, i * 20:(i + 1) * 20] = np.asarray(W1, np.float32)
        W1d[GRP * DM, i * 20:(i + 1) * 20] = np.asarray(b1, np.float32)
    W2a = np.zeros([21, 16], np.float32)
    W2a[0:20, 0:10] = np.asarray(W2, np.float32)
    W2a[20, 0:10] = np.asarray(b2, np.float32)
    W3a = np.zeros([11, 16], np.float32)
    W3a[0:10, 0:1] = np.asarray(W3, np.float32)
    W3a[10, 0:1] = np.asarray(b3, np.float32)
    return (Wpq.astype(bf16), Wc.astype(f8), W1d.astype(bf16), W2a, W3a)


def host_prep_main(plan, PQ_full, edge_attr, Wc8, W1d, W2a, W3a):
    """PQ_full: [60, N] f32 (P rows 0-29, Q' rows 30-59)."""
    nt, off, tb, nblk = plan["nt"], plan["off"], plan["tb"], plan["nblk"]
    src_s, dst_s, rank = plan["src_s"], plan["dst_s"], plan["rank"]
    batch = plan["batch"]

    PQsum = (PQ_full[0:30, src_s] + PQ_full[30:60, dst_s]).astype(f8)  # [30,E]
    ea8 = np.asarray(edge_attr, np.float32).astype(f8)  # [E, 64]
    ident = np.eye(128, dtype=np.float32)

    in_maps = []
    for cd in plan["cores"]:
        lo, hi = cd["lo"], cd["hi"]
        e0 = np.searchsorted(dst_s, lo)
        e1 = np.searchsorted(dst_s, hi)
        lane = cd["lane_of"][dst_s[e0:e1] - lo]
        blk = lane >> 7
        slot = (off[blk] + rank[e0:e1]) * 128 + (lane & 127)
        M = np.zeros([KS, nt * 128], f8)
        M[0:30, slot] = PQsum[:, e0:e1]
        M[30:94, slot] = ea8[plan["order_e"][e0:e1]].T
        gidx = np.full([128, nblk * 2], -1, np.int16)
        oi = cd["oi"]
        pos = np.arange(len(oi))[oi >= 0]
        rel = (batch[lo + oi[oi >= 0]] - cd["glo"]).astype(np.int16)
        gidx[pos & 127, (pos >> 7) * 2] = rel
        in_maps.append({
            "sM": M, "Wc": Wc8, "W1d": W1d, "W2a": W2a, "W3a": W3a,
            "ident": ident, "gidx": gidx,
        })
    return in_maps


# ---------------------------------------------------------------- kernel

_CACHE = {}


def _get_pq_program(npc_pad):
    key = ("pq", npc_pad)
    if key not in _CACHE:
        _CACHE[key] = build_pq_program(npc_pad)
    return _CACHE[key]


def _get_main_program(nblk, tb, ngmax):
    key = ("main", nblk, tuple(tb), ngmax)
    if key not in _CACHE:
        _CACHE[key] = build_main_program(nblk, tuple(int(t) for t in tb), ngmax)
    return _CACHE[key]


last_exec_ns = None
last_exec_ns_pq = None


def kernel(edge_index, node_attr, edge_attr, batch,
           W_msg, b_msg, W1, b1, W2, b2, W3, b3):
    import os
    from concourse.bass_utils import run_bass_kernel_spmd

    global last_exec_ns, last_exec_ns_pq
    trace = bool(os.environ.get("GNN_TRACE"))

    N, D = node_attr.shape
    G = int(np.asarray(batch).max()) + 1 if batch is not None else 0
    # keep G robust: batch is sorted; use max+1 but at least NCORES
    G = max(G, NCORES)

    plan = host_plan(edge_index, batch, N, G)
    max_nc = max(cd["hi"] - cd["lo"] for cd in plan["cores"])
    npc_pad = ((max_nc + 511) // 512) * 512

    nc_pq = _get_pq_program(npc_pad)
    pq_maps = host_prep_pq(plan, node_attr, npc_pad)
    Wpq, Wc8, W1d, W2a, W3a = make_weights(W_msg, b_msg, W1, b1, W2, b2, W3, b3)
    for m in pq_maps:
        m["Wpq"] = Wpq
    res1 = run_bass_kernel_spmd(nc_pq, pq_maps, list(range(NCORES)),
                                trace=trace)
    last_exec_ns_pq = res1.exec_time_ns

    PQ_full = np.zeros([60, N], np.float32)
    for c, cd in enumerate(plan["cores"]):
        lo, hi = cd["lo"], cd["hi"]
        PQ_full[:, lo:hi] = np.asarray(
            res1.results[c]["PQT"]).astype(np.float32)[0:60, :hi - lo]

    nc_main = _get_main_program(plan["nblk"], plan["tb"], plan["ngmax"])
    in_maps = host_prep_main(plan, PQ_full, edge_attr, Wc8, W1d, W2a, W3a)
    res = run_bass_kernel_spmd(nc_main, in_maps, list(range(NCORES)),
                               trace=trace)
    last_exec_ns = res.exec_time_ns

    outv = np.zeros([G, 1], np.float32)
    for c, cd in enumerate(plan["cores"]):
        glo, ghi = cd["glo"], cd["ghi"]
        outv[glo:ghi, 0] = np.asarray(res.results[c]["out"])[0, :ghi - glo]
    return outv


# revision 57
# speedup vs baseline: 1.1830x; 1.1830x over previous
"""GNN message-passing kernel for 8 trn2 NeuronCores (Bass/Tile).

Model (reference):
    msg  = relu(concat(x[src], x[dst], e_attr) @ W_msg + b_msg)   # [E, 30]
    x1   = segment_sum(msg, dst, N)                                # [N, 30]
    h    = relu(x1 @ W1 + b1)                                      # [N, 20]
    g    = segment_sum(h, batch, G)                                # [G, 20]
    out  = relu(g @ W2 + b2) @ W3 + b3                             # [G, 1]

Scheme (v2, "lane-aligned blocks"):
  Cores own whole graphs (G/8 graphs each -> contiguous node range, no
  cross-core node sharing, hence NO collectives).  Within a core, its
  nodes are sorted by in-degree and packed into blocks of 128 "lanes"
  in a "mountain" order (small blocks first and last, biggest in the
  middle, for a fast pipeline start and a short tail).  Block b gets
  T_b = max in-block degree tiles; tile t holds edge #t of every lane
  (zero-padded columns produce msg == 0 because the bias is folded
  into the dst-side node projection).  The per-edge matmul output
  partition therefore IS the destination node: no one-hot scatter
  matmuls, no gpsimd one-hot builds.  relu runs on the scalar (5/6)
  and vector (1/6) engines in 17-tile chunks; the per-node sum is a
  single DVE tensor_reduce per block.

  pass 1: P|Q' = [node_attr^T; 1]^T @ [W_src | W_dst; 0 | b]  (per-core
          node shard, streamed orientation: W stationary).
  host:   gathers P[src] + Q'[dst] (the "replicated node table" gather
          of the sharding strategy), sums them, and assembles the fp8
          per-edge stream sM [96, slots]: rows 0-29 P+Q', 30-93 e_attr.
  pass 2: per block: T_b msg matmuls (fp8 lhsT stream tile, rhs = Wc =
          [I30; W_e]) -> relu -> DVE reduce over tiles -> x1.  Per 4
          blocks: one PE transpose + one block-diagonal W1 matmul gives
          h for 512 nodes; per block one tiny one-hot (gpsimd
          local_scatter, 128 idxs) pools h into a per-core PSUM-resident
          gT [20, 128 graphs].  The graph head runs transposed (gT ->
          W2 -> W3) with no transposes and the core writes its own 125
          graphs; the host concatenates.
"""
import sys

if "/opt/trn_rl_repo" not in sys.path:
    sys.path.insert(0, "/opt/trn_rl_repo")

import numpy as np
import ml_dtypes

bf16 = ml_dtypes.bfloat16
f8 = ml_dtypes.float8_e4m3

NCORES = 8
DM = 30          # message dim
KS = 96          # stream rows (30 PQ + 64 e_attr + 2 pad)
CHT = 17         # msg tiles per relu chunk (17*30 f32 = 2040B <= 2KB bank)
GRP = 4          # blocks per W1 group


# ---------------------------------------------------------------- pass 1

def build_pq_program(npc_pad):
    import concourse.bacc as bacc
    import concourse.mybir as mybir
    import concourse.tile as tile
    from contextlib import ExitStack

    f32, bft = mybir.dt.float32, mybir.dt.bfloat16
    COPY = mybir.ActivationFunctionType.Copy
    NCH = npc_pad // 512

    nc = bacc.Bacc("TRN2", target_bir_lowering=False, debug=True)
    naT = nc.declare_dram_parameter("naT", [65, npc_pad], mybir.dt.float8e4, isOutput=False)
    Wpq = nc.declare_dram_parameter("Wpq", [65, 60], bft, isOutput=False)
    PQT = nc.declare_dram_parameter("PQT", [60, npc_pad], bft, isOutput=True)

    with tile.TileContext(nc) as tc, ExitStack() as xs:
        cp = xs.enter_context(tc.tile_pool(name="const", bufs=1))
        inp = xs.enter_context(tc.tile_pool(name="inp", bufs=16))
        outp = xs.enter_context(tc.tile_pool(name="outp", bufs=3))
        ps = xs.enter_context(tc.tile_pool(name="ps", bufs=3, space="PSUM"))

        Wpq_t = cp.tile([65, 60], bft)
        nc.scalar.dma_start(out=Wpq_t[:], in_=Wpq[:])
        # input chunks: small first chunk so the first matmul starts early
        bounds = [0, 512]
        while bounds[-1] < npc_pad:
            bounds.append(min(npc_pad, bounds[-1] + 1024))
        in_tiles = []
        dmae1 = [nc.sync, nc.gpsimd, nc.scalar]
        for i in range(len(bounds) - 1):
            lo, hi = bounds[i], bounds[i + 1]
            t = inp.tile([65, hi - lo], mybir.dt.float8e4, tag="nat")
            dmae1[i % 3].dma_start(out=t[:], in_=naT[:, lo:hi])
            in_tiles.append((t, lo, hi))

        def find_tile(lo):
            for t, tlo, thi in in_tiles:
                if tlo <= lo < thi:
                    return t, tlo, thi
            raise AssertionError(lo)

        pend = None
        for c in range(NCH):
            lo = c * 512
            ti, tlo, thi = find_tile(lo)
            pq_ps = ps.tile([60, 512], f32, tag="pq")
            nc.tensor.matmul(pq_ps[:], lhsT=Wpq_t[:],
                             rhs=ti[:, lo - tlo:lo - tlo + 512],
                             start=True, stop=True)
            if c % 2 == 0:
                ot = outp.tile([60, 1024], bft, tag="pqt")
                nc.scalar.activation(ot[:, 0:512], pq_ps[:], COPY)
                pend = (ot, lo)
            else:
                ot, olo = pend
                nc.vector.tensor_copy(out=ot[:, 512:1024], in_=pq_ps[:])
                eng = nc.sync if c % 4 == 1 else nc.gpsimd
                eng.dma_start(out=PQT[:, olo:olo + 1024], in_=ot[:])
                pend = None
        if pend is not None:
            ot, olo = pend
            nc.sync.dma_start(out=PQT[:, olo:olo + 512], in_=ot[:, 0:512])
    nc.finalize()
    return nc


# ---------------------------------------------------------------- pass 2

def build_main_program(nblk, tb, ngmax):
    """nblk: padded block count (multiple of GRP); tb: per-block tile
    counts (uniform across cores); ngmax: graphs per core (<=128)."""
    import concourse.bacc as bacc
    import concourse.mybir as mybir
    import concourse.tile as tile
    from contextlib import ExitStack

    f32, bft, fp8 = mybir.dt.float32, mybir.dt.bfloat16, mybir.dt.float8e4
    RELU = mybir.ActivationFunctionType.Relu
    COPY = mybir.ActivationFunctionType.Copy
    MAX = mybir.AluOpType.max
    ADD = mybir.AluOpType.add
    AXX = mybir.AxisListType.X

    nt = sum(tb)
    off = np.zeros(len(tb) + 1, np.int64)
    off[1:] = np.cumsum(tb)

    nc = bacc.Bacc("TRN2", target_bir_lowering=False, debug=True)
    sM = nc.declare_dram_parameter("sM", [KS, nt * 128], fp8, isOutput=False)
    Wc = nc.declare_dram_parameter("Wc", [KS, DM], fp8, isOutput=False)
    W1d = nc.declare_dram_parameter("W1d", [GRP * DM + 1, GRP * 20], bft,
                                    isOutput=False)
    W2a = nc.declare_dram_parameter("W2a", [21, 16], f32, isOutput=False)
    W3a = nc.declare_dram_parameter("W3a", [11, 16], f32, isOutput=False)
    ident = nc.declare_dram_parameter("ident", [128, 128], f32, isOutput=False)
    gidx = nc.declare_dram_parameter("gidx", [128, nblk * 2], mybir.dt.int16,
                                     isOutput=False)
    out = nc.declare_dram_parameter("out", [1, 128], f32, isOutput=True)

    with tile.TileContext(nc) as tc, ExitStack() as xs:
        cp = xs.enter_context(tc.tile_pool(name="const", bufs=1))
        sMp = xs.enter_context(tc.tile_pool(name="sMp", bufs=4))
        msgp = xs.enter_context(tc.tile_pool(name="msgp", bufs=3))
        x14p = xs.enter_context(tc.tile_pool(name="x14p", bufs=3))
        xTap = xs.enter_context(tc.tile_pool(name="xTap", bufs=2))
        h4p = xs.enter_context(tc.tile_pool(name="h4p", bufs=2))
        ohp = xs.enter_context(tc.tile_pool(name="ohp", bufs=2))
        ps_m = xs.enter_context(tc.tile_pool(name="ps_m", bufs=3, space="PSUM"))
        ps_t = xs.enter_context(tc.tile_pool(name="ps_t", bufs=2, space="PSUM"))
        ps_h = xs.enter_context(tc.tile_pool(name="ps_h", bufs=2, space="PSUM"))
        ps_g = xs.enter_context(tc.tile_pool(name="ps_g", bufs=1, space="PSUM"))

        # ---- constants
        Wc_t = cp.tile([KS, DM], fp8)
        nc.sync.dma_start(out=Wc_t[:], in_=Wc[:])
        W1d_t = cp.tile([GRP * DM + 1, GRP * 20], bft)
        nc.scalar.dma_start(out=W1d_t[:], in_=W1d[:])
        W2a_t = cp.tile([21, 16], f32)
        nc.scalar.dma_start(out=W2a_t[:], in_=W2a[:])
        W3a_t = cp.tile([11, 16], f32)
        nc.scalar.dma_start(out=W3a_t[:], in_=W3a[:])
        ident_t = cp.tile([128, 128], f32)
        nc.scalar.dma_start(out=ident_t[:], in_=ident[:])
        gidx_t = cp.tile([128, nblk * 2], mybir.dt.int16)
        nc.scalar.dma_start(out=gidx_t[:], in_=gidx[:])
        ones2 = cp.tile([128, 2], bft)
        nc.vector.memset(ones2[:], 1.0)

        gT_ps = ps_g.tile([20, 128], f32, tag="gT")

        dmae = [nc.sync, nc.gpsimd, nc.scalar]
        ngrp = nblk // GRP
        relu_i = 0
        dma_i = 0
        for g in range(ngrp):
            b0 = g * GRP
            gtiles = int(off[b0 + GRP] - off[b0])
            if gtiles > 0:
                smb = sMp.tile([KS, gtiles * 128], fp8, tag="sM")
                if g < 2:
                    # per-block DMAs so the first matmuls start early
                    for i in range(GRP):
                        c0 = int(off[b0 + i] - off[b0]) * 128
                        c1 = int(off[b0 + i + 1] - off[b0]) * 128
                        if c1 > c0:
                            dmae[dma_i % 2].dma_start(
                                out=smb[:, c0:c1],
                                in_=sM[:, int(off[b0]) * 128 + c0:
                                       int(off[b0]) * 128 + c1])
                            dma_i += 1
                else:
                    glo = int(off[b0]) * 128
                    ghi = int(off[b0 + GRP]) * 128
                    dmae[dma_i % 3].dma_start(out=smb[:], in_=sM[:, glo:ghi])
                    dma_i += 1
            x14 = x14p.tile([128, GRP * DM + 1], f32, tag="x14")
            nc.gpsimd.memset(x14[:, GRP * DM:GRP * DM + 1], 1.0)
            for i in range(GRP):
                b = b0 + i
                T = tb[b]
                if T == 0:
                    nc.gpsimd.memset(x14[:, i * DM:(i + 1) * DM], 0.0)
                    continue
                loc = int(off[b] - off[b0]) * 128
                x14s = x14[:, i * DM:(i + 1) * DM]
                msgblk = msgp.tile([128, T * DM], bft, tag="msg")
                for c0 in range(0, T, CHT):
                    cn = min(CHT, T - c0)
                    mps = ps_m.tile([128, cn * DM], f32, tag="mps")
                    for t in range(cn):
                        e0 = loc + (c0 + t) * 128
                        nc.tensor.matmul(
                            mps[:, t * DM:(t + 1) * DM],
                            lhsT=smb[:, e0:e0 + 128],
                            rhs=Wc_t[:], start=True, stop=True)
                    dst = msgblk[:, c0 * DM:(c0 + cn) * DM]
                    if relu_i % 6 < 5:
                        nc.scalar.activation(dst, mps[:], RELU)
                    else:
                        nc.vector.tensor_scalar(
                            out=dst, in0=mps[:], scalar1=0.0, scalar2=None,
                            op0=MAX)
                    relu_i += 1
                nc.vector.tensor_reduce(
                    out=x14s,
                    in_=msgblk[:].rearrange("p (t d) -> p d t", d=DM),
                    axis=AXX, op=ADD)
            # ---- W1 for the 4 blocks
            xT_ps = ps_t.tile([GRP * DM + 1, 128], f32, tag="xT")
            nc.tensor.transpose(out=xT_ps[:], in_=x14[:], identity=ident_t[:])
            xTa = xTap.tile([GRP * DM + 1, 128], bft, tag="xTa")
            nc.scalar.activation(xTa[:], xT_ps[:], COPY)
            h4_ps = ps_h.tile([128, GRP * 20], f32, tag="h4")
            nc.tensor.matmul(h4_ps[:], lhsT=xTa[:], rhs=W1d_t[:],
                             start=True, stop=True)
            h4 = h4p.tile([128, GRP * 20], bft, tag="h4s")
            nc.scalar.activation(h4[:], h4_ps[:], RELU)
            # ---- pool each block into gT
            for i in range(GRP):
                b = b0 + i
                oh = ohp.tile([128, 128], bft, tag="oh")
                nc.gpsimd.local_scatter(
                    out_ap=oh[:], data_ap=ones2[:],
                    idxs_ap=gidx_t[:, b * 2:b * 2 + 2],
                    channels=128, num_elems=128, num_idxs=2)
                nc.tensor.matmul(gT_ps[:], lhsT=h4[:, i * 20:(i + 1) * 20],
                                 rhs=oh[:], start=(b == 0), stop=(b == nblk - 1))

        # ---- graph head (per-core, transposed; no collectives)
        gTa = cp.tile([21, 128], f32)
        nc.vector.memset(gTa[:], 1.0)
        nc.vector.tensor_copy(out=gTa[:20, :], in_=gT_ps[:])
        r_ps = ps_m.tile([16, 128], f32, tag="mps")
        nc.tensor.matmul(r_ps[:], lhsT=W2a_t[:], rhs=gTa[:], start=True,
                         stop=True)
        rTa = cp.tile([11, 128], f32)
        nc.vector.memset(rTa[:], 1.0)
        nc.scalar.activation(rTa[:10, :], r_ps[:10, :], RELU)
        o_ps = ps_h.tile([16, 128], f32, tag="h4")
        nc.tensor.matmul(o_ps[:], lhsT=W3a_t[:], rhs=rTa[:], start=True,
                         stop=True)
        outsb = cp.tile([1, 128], f32)
        nc.vector.tensor_copy(out=outsb[:], in_=o_ps[:1, :])
        nc.sync.dma_start(out=out[:], in_=outsb[:])

    nc.finalize()
    return nc


# ---------------------------------------------------------------- host prep

def host_plan(edge_index, batch, n_nodes, n_graphs):
    """Graph-aligned core ownership + degree-sorted lane blocks."""
    src = np.asarray(edge_index[0]).astype(np.int64)
    dst = np.asarray(edge_index[1]).astype(np.int64)
    batch = np.asarray(batch).astype(np.int64)
    N, G = n_nodes, n_graphs

    gcnt = np.bincount(batch, minlength=G)
    gstart = np.zeros(G + 1, np.int64)
    gstart[1:] = np.cumsum(gcnt)
    glo = [c * G // NCORES for c in range(NCORES + 1)]
    nlo = [int(gstart[glo[c]]) for c in range(NCORES + 1)]

    deg = np.bincount(dst, minlength=N)

    order_e = np.argsort(dst, kind="stable")
    dst_s = dst[order_e]
    src_s = src[order_e]
    # per-edge rank within its dst run
    run_start = np.zeros(len(dst_s), np.int64)
    newrun = np.r_[True, dst_s[1:] != dst_s[:-1]]
    idx = np.arange(len(dst_s))
    run_start = np.maximum.accumulate(np.where(newrun, idx, 0))
    rank = idx - run_start

    # per-core sorted lanes
    cores = []
    nblk_max = 0
    for c in range(NCORES):
        lo, hi = nlo[c], nlo[c + 1]
        dg = deg[lo:hi]
        # "mountain" block order: small blocks first (fast pipeline
        # start) and last (short tail), biggest in the middle.  `oi` maps
        # lane position -> local node index (-1 = hole/pad lane).
        order_n = np.argsort(dg, kind="stable")
        nblk = (hi - lo + 127) // 128
        pad_n = nblk * 128 - (hi - lo)
        oi = np.concatenate([order_n, np.full(pad_n, -1, np.int64)])
        blocks = [oi[k * 128:(k + 1) * 128] for k in range(nblk)]
        oi = np.concatenate(blocks[0::2] + blocks[1::2][::-1])
        lane_of = np.full(hi - lo, -1, np.int64)
        pos = np.arange(nblk * 128)
        lane_of[oi[oi >= 0]] = pos[oi >= 0]
        nblk_max = max(nblk_max, nblk)
        dgp = np.where(oi >= 0, deg[lo + np.maximum(oi, 0)], 0)
        tbc = dgp.reshape(nblk, 128).max(axis=1)
        cores.append(dict(lo=lo, hi=hi, oi=oi, lane_of=lane_of,
                          tbc=tbc, glo=glo[c], ghi=glo[c + 1]))

    nblk = ((nblk_max + GRP - 1) // GRP) * GRP
    tb = np.zeros(nblk, np.int64)
    for cd in cores:
        tb[:len(cd["tbc"])] = np.maximum(tb[:len(cd["tbc"])], cd["tbc"])
    off = np.zeros(nblk + 1, np.int64)
    off[1:] = np.cumsum(tb)

    ngmax = max(cd["ghi"] - cd["glo"] for cd in cores)
    assert ngmax <= 128

    return dict(cores=cores, nblk=nblk, tb=tb, off=off, nt=int(tb.sum()),
                order_e=order_e, dst_s=dst_s, src_s=src_s, rank=rank,
                deg=deg, ngmax=ngmax, batch=batch)


def host_prep_pq(plan, node_attr, npc_pad):
    na = np.asarray(node_attr, np.float32)
    in_maps = []
    for cd in plan["cores"]:
        lo, hi = cd["lo"], cd["hi"]
        naT = np.zeros([65, npc_pad], f8)
        naT[0:64, :hi - lo] = na[lo:hi].T.astype(f8)
        naT[64, :] = f8(1.0)
        in_maps.append({"naT": naT})
    return in_maps


def make_weights(W_msg, b_msg, W1, b1, W2, b2, W3, b3):
    W_msg = np.asarray(W_msg, np.float32)
    Wpq = np.zeros([65, 60], np.float32)
    Wpq[0:64, 0:DM] = W_msg[0:64]
    Wpq[0:64, DM:2 * DM] = W_msg[64:128]
    Wpq[64, DM:2 * DM] = np.asarray(b_msg, np.float32)
    Wc = np.zeros([KS, DM], np.float32)
    Wc[0:DM] = np.eye(DM)
    Wc[DM:DM + 64] = W_msg[128:192]
    W1d = np.zeros([GRP * DM + 1, GRP * 20], np.float32)
    for i in range(GRP):
        W1d[i * DM:(i + 1) * DM, i * 20:(i + 1) * 20] = np.asarray(W1, np.float32)
        W1d[GRP * DM, i * 20:(i + 1) * 20] = np.asarray(b1, np.float32)
    W2a = np.zeros([21, 16], np.float32)
    W2a[0:20, 0:10] = np.asarray(W2, np.float32)
    W2a[20, 0:10] = np.asarray(b2, np.float32)
    W3a = np.zeros([11, 16], np.float32)
    W3a[0:10, 0:1] = np.asarray(W3, np.float32)
    W3a[10, 0:1] = np.asarray(b3, np.float32)
    return (Wpq.astype(bf16), Wc.astype(f8), W1d.astype(bf16), W2a, W3a)


def host_prep_main(plan, PQ_full, edge_attr, Wc8, W1d, W2a, W3a):
    """PQ_full: [60, N] f32 (P rows 0-29, Q' rows 30-59)."""
    nt, off, tb, nblk = plan["nt"], plan["off"], plan["tb"], plan["nblk"]
    src_s, dst_s, rank = plan["src_s"], plan["dst_s"], plan["rank"]
    batch = plan["batch"]

    PQsum = (PQ_full[0:30, src_s] + PQ_full[30:60, dst_s]).astype(f8)  # [30,E]
    ea8 = np.asarray(edge_attr, np.float32).astype(f8)  # [E, 64]
    ident = np.eye(128, dtype=np.float32)

    in_maps = []
    for cd in plan["cores"]:
        lo, hi = cd["lo"], cd["hi"]
        e0 = np.searchsorted(dst_s, lo)
        e1 = np.searchsorted(dst_s, hi)
        lane = cd["lane_of"][dst_s[e0:e1] - lo]
        blk = lane >> 7
        slot = (off[blk] + rank[e0:e1]) * 128 + (lane & 127)
        M = np.zeros([KS, nt * 128], f8)
        M[0:30, slot] = PQsum[:, e0:e1]
        M[30:94, slot] = ea8[plan["order_e"][e0:e1]].T
        gidx = np.full([128, nblk * 2], -1, np.int16)
        oi = cd["oi"]
        pos = np.arange(len(oi))[oi >= 0]
        rel = (batch[lo + oi[oi >= 0]] - cd["glo"]).astype(np.int16)
        gidx[pos & 127, (pos >> 7) * 2] = rel
        in_maps.append({
            "sM": M, "Wc": Wc8, "W1d": W1d, "W2a": W2a, "W3a": W3a,
            "ident": ident, "gidx": gidx,
        })
    return in_maps


# ---------------------------------------------------------------- kernel

_CACHE = {}


def _get_pq_program(npc_pad):
    key = ("pq", npc_pad)
    if key not in _CACHE:
        _CACHE[key] = build_pq_program(npc_pad)
    return _CACHE[key]


def _get_main_program(nblk, tb, ngmax):
    key = ("main", nblk, tuple(tb), ngmax)
    if key not in _CACHE:
        _CACHE[key] = build_main_program(nblk, tuple(int(t) for t in tb), ngmax)
    return _CACHE[key]


last_exec_ns = None
last_exec_ns_pq = None


def kernel(edge_index, node_attr, edge_attr, batch,
           W_msg, b_msg, W1, b1, W2, b2, W3, b3):
    import os
    from concourse.bass_utils import run_bass_kernel_spmd

    global last_exec_ns, last_exec_ns_pq
    trace = bool(os.environ.get("GNN_TRACE"))

    N, D = node_attr.shape
    G = int(np.asarray(batch).max()) + 1 if batch is not None else 0
    # keep G robust: batch is sorted; use max+1 but at least NCORES
    G = max(G, NCORES)

    plan = host_plan(edge_index, batch, N, G)
    max_nc = max(cd["hi"] - cd["lo"] for cd in plan["cores"])
    npc_pad = ((max_nc + 511) // 512) * 512

    nc_pq = _get_pq_program(npc_pad)
    pq_maps = host_prep_pq(plan, node_attr, npc_pad)
    Wpq, Wc8, W1d, W2a, W3a = make_weights(W_msg, b_msg, W1, b1, W2, b2, W3, b3)
    for m in pq_maps:
        m["Wpq"] = Wpq
    res1 = run_bass_kernel_spmd(nc_pq, pq_maps, list(range(NCORES)),
                                trace=trace)
    last_exec_ns_pq = res1.exec_time_ns

    PQ_full = np.zeros([60, N], np.float32)
    for c, cd in enumerate(plan["cores"]):
        lo, hi = cd["lo"], cd["hi"]
        PQ_full[:, lo:hi] = np.asarray(
            res1.results[c]["PQT"]).astype(np.float32)[0:60, :hi - lo]

    nc_main = _get_main_program(plan["nblk"], plan["tb"], plan["ngmax"])
    in_maps = host_prep_main(plan, PQ_full, edge_attr, Wc8, W1d, W2a, W3a)
    res = run_bass_kernel_spmd(nc_main, in_maps, list(range(NCORES)),
                               trace=trace)
    last_exec_ns = res.exec_time_ns

    outv = np.zeros([G, 1], np.float32)
    for c, cd in enumerate(plan["cores"]):
        glo, ghi = cd["glo"], cd["ghi"]
        outv[glo:ghi, 0] = np.asarray(res.results[c]["out"])[0, :ghi - glo]
    return outv
